# revision 16
# baseline (speedup 1.0000x reference)
"""CTGRU cell kernel for Trainium2, data-parallel across 8 NeuronCores.

Math (per batch row; U=512 units, M=8 tau-scales):
  h_hat = state.reshape(U, M); h = sum_m h_hat
  fused = [inputs, h]                                   # [1024]
  r = fused @ Wr;  rki = softmax_m(-(r - tau)^2)
  q = sum_m rki * h_hat
  qk = tanh([inputs, q] @ Wq)
  s = fused @ Ws;  ski = softmax_m(-(s - tau)^2)
  h_hat_next = ((1-ski) h_hat + ski qk) * decay
  h_next = sum_m h_hat_next

Device layout: FEATURE-major ([feat, batch]) so every per-(u,m) constant
(tau, decay) is a per-partition scalar fused into ACT affine inputs, and
all m-group reductions/broadcasts are PE matmuls against 0/1 matrices.
Main matmuls run bf16 (weights/activations bf16, fp32 PSUM accumulate);
aux 0/1-matrix matmuls run float32r on the f32 elementwise tensors.
Aux matmuls are software-pipelined 1-2 ticks behind the producing
ACT/DVE chain so the PE never stalls on them.
"""
import numpy as np

B, I, U, M = 4096, 512, 512, 8
NCORES = 8
BL = B // NCORES  # 512 batch rows per core
NJ = U * M // 128  # 32 feature tiles of 128
NA = U // 128  # 4 unit tiles of 128

_CACHE = {}


# ---------------------------------------------------------------- constants
def _consts():
    tau = (np.arange(M) * 0.5 * np.log(10.0)).astype(np.float64)
    with np.errstate(divide="ignore"):
        decay = np.exp(-1.0 / tau)
    decay[0] = 0.0
    p = np.arange(128)
    neg_tau_p = (-tau[p % 8]).astype(np.float32).reshape(128, 1)
    decay_p = decay[p % 8].astype(np.float32).reshape(128, 1)
    # A[k][f, u] = 1 iff u == 16k + f//8 : lhsT summing m-groups of 8
    # partitions (feature space f = 8*u_local + m) into unit space.
    A = np.zeros((8, 128, 128), np.float32)
    for k in range(8):
        A[k, np.arange(128), 16 * k + np.arange(128) // 8] = 1.0
    # BD[f1, f2] = 1 iff f1//8 == f2//8 : replicated m-group sums.
    BD = np.zeros((128, 128), np.float32)
    for f in range(128):
        BD[f, (f // 8) * 8 : (f // 8) * 8 + 8] = 1.0
    return neg_tau_p, decay_p, A, BD


# ---------------------------------------------------------------- device program
def build_nc(reps=1, mode="full", pool_offload=True, v4=False,
             v6=False):
    import ml_dtypes

    import concourse.bacc as bacc
    import concourse.mybir as mybir
    import concourse.tile as tile

    f32 = mybir.dt.float32
    f32r = mybir.dt.float32r
    bf16 = mybir.dt.bfloat16

    def r(ap):  # float32r view for aux-matmul operands
        return ap.bitcast(f32r)

    neg_tau_p, decay_p, A_np, BD_np = _consts()

    nc = bacc.Bacc("TRN2", target_bir_lowering=False, debug=False)
    d_inT = nc.dram_tensor("inputsT", [I, BL], bf16, kind="ExternalInput")
    d_stT = nc.dram_tensor("stateT", [U * M, BL], bf16, kind="ExternalInput")
    d_wr = nc.dram_tensor("Wr", [I + U, U * M], bf16, kind="ExternalInput")
    d_ws = nc.dram_tensor("Ws", [I + U, U * M], bf16, kind="ExternalInput")
    d_wq = nc.dram_tensor("Wq", [I + U, U], bf16, kind="ExternalInput")
    d_hh = nc.dram_tensor("hhT", [U * M, BL], f32, kind="ExternalOutput")
    d_hn = nc.dram_tensor("hnT", [U, BL], f32, kind="ExternalOutput")

    A2 = np.ascontiguousarray(A_np.reshape(8 * 128, 128))
    bf = ml_dtypes.bfloat16
    d_Ab = nc.inline_tensor(A2.astype(bf), name="constAb")
    AD = A_np * decay_p.reshape(1, 128, 1)  # decay folded into h_next reduce
    d_AD = nc.inline_tensor(
        np.ascontiguousarray(AD.reshape(8 * 128, 128)).astype(bf), name="constAD"
    )
    d_B = nc.inline_tensor(
        np.ascontiguousarray(
            np.transpose(A_np, (0, 2, 1)).reshape(8 * 128, 128)
        ).astype(bf),
        name="constB",
    )
    d_BD = nc.inline_tensor(BD_np.astype(bf), name="constBD")
    d_nI = nc.inline_tensor((-np.eye(128, dtype=np.float32)).astype(bf),
                            name="constnI")
    d_ntau = nc.inline_tensor(neg_tau_p, name="constNtau")
    d_dec = nc.inline_tensor(decay_p, name="constDec")

    Sq = mybir.ActivationFunctionType.Square
    Ex = mybir.ActivationFunctionType.Exp
    Th = mybir.ActivationFunctionType.Tanh
    mult = mybir.AluOpType.mult
    sub = mybir.AluOpType.subtract
    add = mybir.AluOpType.add

    def packed(dram_ap, n):
        """[n*128, C] dram slab -> [128, n, C] AP (partition-packed)."""
        return dram_ap.rearrange("(a p) c -> p a c", p=128)

    with tile.TileContext(nc) as tc:
        with (
            tc.tile_pool(name="cpool", bufs=1) as cpool,
            tc.tile_pool(name="spool", bufs=1) as spool,
            tc.tile_pool(name="fpool", bufs=1) as fpool,
            tc.tile_pool(name="wpool", bufs=10) as wpool,
            tc.tile_pool(name="tpool", bufs=2) as tpool,
            tc.tile_pool(name="opool", bufs=3) as opool,
            tc.tile_pool(name="pmain", bufs=3, space="PSUM") as pmain,
            tc.tile_pool(name="paux", bufs=3, space="PSUM") as paux,
            tc.tile_pool(name="pacc", bufs=2, space="PSUM") as pacc,
        ):
            # ---- constants to SBUF (slab-packed, one DMA each; all bf16)
            abslab = cpool.tile([128, 8 * 128], bf16, name="abslab")
            nc.sync.dma_start(
                abslab[:].rearrange("p (a c) -> p a c", a=8), packed(d_Ab[:, :], 8)
            )
            adslab = cpool.tile([128, 8 * 128], bf16, name="adslab")
            nc.sync.dma_start(
                adslab[:].rearrange("p (a c) -> p a c", a=8), packed(d_AD[:, :], 8)
            )
            bslab = cpool.tile([128, 8 * 128], bf16, name="bslab")
            nc.sync.dma_start(
                bslab[:].rearrange("p (a c) -> p a c", a=8), packed(d_B[:, :], 8)
            )
            bd_t = cpool.tile([128, 128], bf16, name="bd")
            nc.sync.dma_start(bd_t[:], d_BD[:, :])
            ni_t = cpool.tile([128, 128], bf16, name="ni")
            nc.sync.dma_start(ni_t[:], d_nI[:, :])
            ntau = cpool.tile([128, 1], f32, name="ntau")
            nc.sync.dma_start(ntau[:], d_ntau[:, :])
            dec = cpool.tile([128, 1], f32, name="dec")
            nc.sync.dma_start(dec[:], d_dec[:, :])
            a_t = [abslab[:, 128 * k : 128 * (k + 1)] for k in range(8)]
            ab_t = a_t
            ad_t = [adslab[:, 128 * k : 128 * (k + 1)] for k in range(8)]
            b_t = [bslab[:, 128 * k : 128 * (k + 1)] for k in range(8)]

            def load_wgroup(d_w, tag, g):
                """Weight group g (out-cols 1024g..1024g+1024), all 8 k-chunks,
                as 4 slabs [128, 2048] bf16 (2 k-chunks each)."""
                slabs = []
                for s in range(4):
                    w_ = wpool.tile([128, 2048], bf16, name=f"{tag}g{g}s{s}",
                                    tag="wch")
                    src = d_w[256 * s : 256 * (s + 1),
                              1024 * g : 1024 * (g + 1)]
                    nc.sync.dma_start(
                        w_[:].rearrange("p (a c) -> p a c", a=2), packed(src, 2)
                    )
                    slabs.append(w_)

                def lhsT(k, c):
                    return slabs[k // 2][:, 1024 * (k % 2) + 128 * c :
                                         1024 * (k % 2) + 128 * (c + 1)]

                return lhsT

            def body():
                do_aux = mode not in ("mm", "dma")
                do_mm = mode != "dma"
                # ---- state (feature-major bf16): 8 slabs of 4 j-tiles
                st = []
                for s in range(8):
                    sl_ = spool.tile([128, 4 * BL], bf16, name=f"sts{s}",
                                     tag=f"sts{s}")
                    src = d_stT[512 * s : 512 * (s + 1), :]
                    nc.sync.dma_start(
                        sl_[:].rearrange("p (a c) -> p a c", a=4), packed(src, 4)
                    )
                    for jj in range(4):
                        st.append(sl_[:, BL * jj : BL * (jj + 1)])
                # ---- inputsT: one slab of 4 k-tiles
                inslab = fpool.tile([128, 4 * BL], bf16, name="inslab")
                nc.sync.dma_start(
                    inslab[:].rearrange("p (a c) -> p a c", a=4),
                    packed(d_inT[:, :], 4),
                )
                F = [inslab[:, BL * k : BL * (k + 1)] for k in range(4)]
                # ---- hT = sum_m h_hat (bf16 MMs); evacuate as bf16
                for a in range(NA):
                    fh = fpool.tile([128, BL], bf16, name=f"fh{a}", tag=f"fh{a}")
                    if do_mm:
                        ph = pacc.tile([128, BL], f32, name=f"ph{a}", tag="acc")
                        for k in range(8):
                            nc.tensor.matmul(
                                ph[:], lhsT=ab_t[k], rhs=st[8 * a + k],
                                start=(k == 0), stop=(k == 7),
                            )
                        nc.scalar.copy(out=fh[:], in_=ph[:])
                    F.append(fh[:])

                # ---- r-phase (software-pipelined aux MMs, depth 2)
                q_t = []
                for a in range(NA):
                    wT = load_wgroup(d_wr, "wr", a)
                    pden = pacc.tile([128, BL], f32, name=f"pden{a}", tag="acc")
                    pnum = pacc.tile([128, BL], f32, name=f"pnum{a}", tag="acc")
                    e_st, w_st = {}, {}
                    for t in range(10):
                        if t < 8:
                            c = t
                            j = 8 * a + c
                            px = pmain.tile([128, BL], f32, name=f"pxr{j}",
                                            tag="px")
                            if do_mm:
                                for k in range(8):
                                    nc.tensor.matmul(
                                        px[:], lhsT=wT(k, c), rhs=F[k],
                                        start=(k == 0), stop=(k == 7),
                                    )
                            if mode in ("full", "act"):
                                sq = tpool.tile([128, BL], f32,
                                                name=f"sqr{j}", tag="sq")
                                nc.scalar.activation(sq[:], px[:], Sq,
                                                     bias=ntau[:, 0:1])
                                e_ = tpool.tile([128, BL], bf16,
                                                name=f"er{j}", tag="er",
                                                bufs=3)
                                nc.scalar.activation(e_[:], sq[:], Ex,
                                                     scale=-1.0)
                                if mode == "full":
                                    w_r = tpool.tile([128, BL], bf16,
                                                     name=f"wrr{j}",
                                                     tag="wrr", bufs=3)
                                    eng = (nc.gpsimd if pool_offload
                                           else nc.vector)
                                    eng.tensor_tensor(w_r[:], e_[:],
                                                      st[j], op=mult)
                                    e_st[c], w_st[c] = e_, w_r
                                else:
                                    e_st[c], w_st[c] = e_, None
                        if 0 <= t - 2 < 8 and do_aux:
                            cc = t - 2
                            rhs_e = (e_st[cc][:] if mode in ("full", "act")
                                     else st[8 * a + cc])
                            rhs_w = (w_st[cc][:] if mode == "full"
                                     else st[8 * a + cc])
                            nc.tensor.matmul(
                                pden[:], lhsT=a_t[cc], rhs=rhs_e,
                                start=(cc == 0), stop=(cc == 7),
                            )
                            nc.tensor.matmul(
                                pnum[:], lhsT=a_t[cc], rhs=rhs_w,
                                start=(cc == 0), stop=(cc == 7),
                            )
                    if mode == "full":
                        inv_r = tpool.tile([128, BL], f32, name=f"invr{a}",
                                           tag="invr", bufs=1)
                        nc.vector.reciprocal_approx_fast(out=inv_r[:], in_=pden[:])
                        q_ = fpool.tile([128, BL], bf16, name=f"q{a}", tag=f"q{a}")
                        nc.vector.tensor_tensor(q_[:], pnum[:], inv_r[:], op=mult)
                        q_t.append(q_[:])
                    else:
                        q_t.append(st[2*a])

                # ---- qk = tanh([inputsT; q] @ Wq)
                wqslab = fpool.tile([128, 8 * U], bf16, name="wqslab")
                nc.sync.dma_start(
                    wqslab[:].rearrange("p (a c) -> p a c", a=8),
                    packed(d_wq[:, :], 8),
                )
                G = F[:4] + q_t
                qk_t = []
                for a in range(NA):
                    pq = paux.tile([128, BL], f32, name=f"pq{a}", tag="aux")
                    for k in range(8 if do_mm else 0):
                        nc.tensor.matmul(
                            pq[:],
                            lhsT=wqslab[:, U * k + 128 * a : U * k + 128 * (a + 1)],
                            rhs=G[k],
                            start=(k == 0), stop=(k == 7),
                        )
                    if mode == "full":
                        qk_ = fpool.tile([128, BL], bf16, name=f"qk{a}",
                                         tag=f"qk{a}")
                        nc.scalar.activation(qk_[:], pq[:], Th)
                        qk_t.append(qk_[:])
                    else:
                        qk_t.append(st[2*a+1])

                # ---- s-phase (pipelined: chain at t-1, h_next MM at t-2)
                for a in range(NA):
                    wT = load_wgroup(d_ws, "ws", a)
                    phn = pacc.tile([128, BL], f32, name=f"phn{a}", tag="acc")
                    e_st, pqr_st, o_st, oslabs = {}, {}, {}, {}
                    for t in range(10):
                        if t < 8:
                            c = t
                            j = 8 * a + c
                            pqr = paux.tile([128, BL], f32, name=f"pqr{j}",
                                            tag="aux")
                            if do_aux:
                                nc.tensor.matmul(pqr[:], lhsT=b_t[c],
                                                 rhs=qk_t[a],
                                                 start=True, stop=not v6)
                                if v6:
                                    nc.tensor.matmul(pqr[:], lhsT=ni_t[:],
                                                     rhs=st[j],
                                                     start=False, stop=True)
                            pqr_st[c] = pqr
                            px = pmain.tile([128, BL], f32, name=f"pxs{j}",
                                            tag="px")
                            if do_mm:
                                for k in range(8):
                                    nc.tensor.matmul(
                                        px[:], lhsT=wT(k, c), rhs=F[k],
                                        start=(k == 0), stop=(k == 7),
                                    )
                            pqr_st[c] = pqr
                            if do_mm:
                                pass
                            if mode in ("full", "act"):
                                sq = tpool.tile([128, BL], f32,
                                                name=f"sqs{j}", tag="sq")
                                nc.scalar.activation(sq[:], px[:], Sq,
                                                     bias=ntau[:, 0:1])
                                e_ = tpool.tile([128, BL], bf16,
                                                name=f"es{j}", tag="es",
                                                bufs=3)
                                nc.scalar.activation(e_[:], sq[:], Ex,
                                                     scale=-1.0)
                                e_st[c] = e_
                        if 0 <= t - 1 < 8:
                            cc = t - 1
                            j1 = 8 * a + cc
                            pdr = paux.tile([128, BL], f32, name=f"pdr{j1}",
                                            tag="aux")
                            rhs_es = (e_st[cc][:] if mode in ("full", "act")
                                      else st[j1])
                            if do_aux:
                                nc.tensor.matmul(pdr[:], lhsT=bd_t[:],
                                                 rhs=rhs_es,
                                                 start=True, stop=True)
                            if mode != "full":
                                o_st[cc] = None
                                pqr_st[cc] = None
                                continue
                            invd = tpool.tile([128, BL], f32, name=f"invd{j1}",
                                              tag="invd")
                            nc.vector.reciprocal_approx_fast(out=invd[:],
                                                             in_=pdr[:])
                            if v6:
                                invdb = tpool.tile([128, BL], bf16,
                                                   name=f"invdb{j1}",
                                                   tag="invdb", bufs=3)
                                nc.scalar.copy(out=invdb[:], in_=invd[:])
                            if cc % 4 == 0:
                                osl = opool.tile([128, 4 * BL], f32,
                                                 name=f"osl{j1}", tag="osl")
                                oslabs[cc // 4] = osl
                            osl = oslabs[cc // 4]
                            o_ = osl[:, BL * (cc % 4) : BL * (cc % 4 + 1)]
                            d_ = tpool.tile([128, BL], bf16, name=f"d{j1}",
                                            tag="d", bufs=3)
                            if v6:
                                # pqr already holds qk_rep - h_hat
                                nc.vector.tensor_tensor(d_[:], e_st[cc][:],
                                                        pqr_st[cc][:], op=mult)
                                nc.vector.tensor_tensor(d_[:], d_[:],
                                                        invdb[:], op=mult)
                            else:
                                nc.vector.tensor_tensor(d_[:], pqr_st[cc][:],
                                                        st[j1], op=sub)
                                nc.vector.tensor_tensor(d_[:], e_st[cc][:],
                                                        d_[:], op=mult)
                                nc.vector.tensor_tensor(d_[:], d_[:],
                                                        invd[:], op=mult)
                            t4 = tpool.tile([128, BL], bf16, name=f"t4{j1}",
                                            tag="t4", bufs=3)
                            eng = nc.gpsimd if pool_offload else nc.vector
                            eng.tensor_tensor(t4[:], st[j1], d_[:], op=add)
                            eng2 = nc.gpsimd if (pool_offload and v4) else nc.vector
                            eng2.tensor_scalar_mul(o_, t4[:], dec[:, 0:1])
                            o_st[cc] = t4
                        if 0 <= t - 2 < 8 and do_aux:
                            cc2 = t - 2
                            rhs_o = (o_st[cc2][:] if mode == "full"
                                     else st[8 * a + cc2])
                            nc.tensor.matmul(
                                phn[:], lhsT=ad_t[cc2], rhs=rhs_o,
                                start=(cc2 == 0), stop=(cc2 == 7),
                            )
                            if cc2 % 4 == 3 and mode == "full":
                                sidx = cc2 // 4
                                j0 = 8 * a + 4 * sidx
                                dst = d_hh[128 * j0 : 128 * (j0 + 4), :]
                                nc.sync.dma_start(
                                    packed(dst, 4),
                                    oslabs[sidx][:].rearrange(
                                        "p (a c) -> p a c", a=4
                                    ),
                                )
                    if mode == "full":
                        hn_ = opool.tile([128, BL], f32, name=f"hn{a}",
                                         tag="hns", bufs=2)
                        nc.scalar.copy(out=hn_[:], in_=phn[:])
                        nc.sync.dma_start(d_hn[128 * a : 128 * (a + 1), :],
                                          hn_[:])

            if reps == 1:
                body()
            else:
                with tc.For_i(0, reps, 1):
                    body()
    nc.compile()
    return nc


# ---------------------------------------------------------------- host glue
def _shard_inputs(inputs, state, Wr, Ws, Wq):
    import ml_dtypes

    bf = ml_dtypes.bfloat16
    stateT = np.ascontiguousarray(state.T).astype(bf)
    inputsT = np.ascontiguousarray(inputs.T).astype(bf)
    Wrb, Wsb, Wqb = Wr.astype(bf), Ws.astype(bf), Wq.astype(bf)
    in_maps = []
    for c in range(NCORES):
        sl = slice(c * BL, (c + 1) * BL)
        in_maps.append(
            {
                "inputsT": np.ascontiguousarray(inputsT[:, sl]),
                "stateT": np.ascontiguousarray(stateT[:, sl]),
                "Wr": Wrb,
                "Ws": Wsb,
                "Wq": Wqb,
            }
        )
    return in_maps


def kernel(inputs, state, Wr, br, Ws, bs, Wq, bq, **_unused):
    """Full inputs in, full outputs out. br/bs/bq are zeros by the problem
    spec (fill: zeros) and are folded out."""
    inputs = np.ascontiguousarray(np.asarray(inputs, dtype=np.float32))
    state = np.ascontiguousarray(np.asarray(state, dtype=np.float32))
    Wr = np.ascontiguousarray(np.asarray(Wr, dtype=np.float32))
    Ws = np.ascontiguousarray(np.asarray(Ws, dtype=np.float32))
    Wq = np.ascontiguousarray(np.asarray(Wq, dtype=np.float32))

    from concourse.bass_utils import run_bass_kernel_spmd

    if "nc" not in _CACHE:
        _CACHE["nc"] = build_nc()
    nc = _CACHE["nc"]
    in_maps = _shard_inputs(inputs, state, Wr, Ws, Wq)
    res = run_bass_kernel_spmd(nc, in_maps, core_ids=list(range(NCORES)))

    h_next = np.empty((B, U), np.float32)
    h_hat_next = np.empty((B, U * M), np.float32)
    for c in range(NCORES):
        sl = slice(c * BL, (c + 1) * BL)
        h_next[sl] = res.results[c]["hnT"].T
        h_hat_next[sl] = res.results[c]["hhT"].T
    return h_next, h_hat_next
